# revision 46
# baseline (speedup 1.0000x reference)
"""Distributed causal multi-head attention block for 8 TRN2 NeuronCores.

Sharding: core i -> (batch b = i//2, head-half hh = i%2).  Each core computes
attention for 6 of the 12 heads of one batch element, then a row-sharded
c_proj (its 384 input channels -> full 768 outputs, partial sums).  The
host sums the two partial projections per batch (the "all-reduce" of the
tensor-parallel c_proj) and adds b_proj.

Everything on-chip lives transposed ([feature, token]) so no transposes are
needed:
  qkT = W_qk @ x^T          (heads' Q^T,K^T in [d, t] layout)
  V   = x @ Wv^T            ([t, d] layout, + per-head ones column)
  S^T = K_blk @ Q_blk^T     -> exp (scale 1/8 folded in) -> causal mask
  O^T_aug = [V|1]^T @ P^T   (row 64 of each head block = softmax denom)
  y^T = O^T * (1/denom)  + bv
  out^T = Wp_half @ y^T     (partial over this core's heads)
"""

import sys

sys.path.insert(0, "/opt/trn_rl_repo")

import numpy as np
import ml_dtypes

import concourse.bass as bass
import concourse.bacc as bacc
import concourse.mybir as mybir
import concourse.tile as tile
from concourse.bass_utils import run_bass_kernel_spmd

BF16 = mybir.dt.bfloat16
F32 = mybir.dt.float32
F32R = mybir.dt.float32r
AF = mybir.ActivationFunctionType
ALU = mybir.AluOpType

B, T, C, H, HD = 4, 2048, 768, 12, 64
NCORES = 8
HH = 6              # heads per core
CH = HH * HD        # 384 channels per core
NCT = C // 128      # 6 contraction tiles over C
NTT = T // 128      # 16 token tiles
NQC = T // 512      # 4 query chunks
VW = 65             # per-head V block width (64 dims + ones column)


def _build_graph():
    nc = bacc.Bacc("TRN2", target_bir_lowering=False)

    xT = nc.declare_dram_parameter("xT", [C, T], BF16, isOutput=False)
    wqkT = nc.declare_dram_parameter("wqkT", [C, 2 * CH], BF16, isOutput=False)
    bqk = nc.declare_dram_parameter("bqk", [128, 2 * CH // 128], F32, isOutput=False)
    wvT = nc.declare_dram_parameter("wvT", [C, CH], BF16, isOutput=False)
    bv = nc.declare_dram_parameter("bv", [128, CH // 128], F32, isOutput=False)
    wpT = nc.declare_dram_parameter("wpT", [CH, C], BF16, isOutput=False)
    masks = nc.declare_dram_parameter("masks", [128, 4 * 1024], BF16, isOutput=False)
    out = nc.declare_dram_parameter("out", [C, T], F32, isOutput=True)

    with tile.TileContext(nc) as tc:
        with (
            tc.tile_pool(name="weights", bufs=1) as wpool,
            tc.tile_pool(name="acts", bufs=1) as apool,
            tc.tile_pool(name="ps2", bufs=2, space="PSUM") as ps2,
            tc.tile_pool(name="pacc", bufs=2, space="PSUM") as pacc,
            tc.tile_pool(name="ptile", bufs=8) as ppool,
            tc.tile_pool(name="small", bufs=6) as spool,
            tc.tile_pool(name="ostage", bufs=6) as opool,
        ):
            # ---- load everything ----
            xT_s = [wpool.tile([128, T], BF16, tag=f"xT{i}", name=f"xT{i}") for i in range(NCT)]
            wqkT_s = [wpool.tile([128, 2 * CH], BF16, tag=f"wqk{i}", name=f"wqk{i}") for i in range(NCT)]
            wvT_s = [wpool.tile([128, CH], BF16, tag=f"wv{i}", name=f"wv{i}") for i in range(NCT)]
            wpT_s = [wpool.tile([128, C], BF16, tag=f"wp{i}", name=f"wp{i}") for i in range(CH // 128)]
            bqk_s = wpool.tile([128, 2 * CH // 128], F32, tag="bqk")
            bv_s = wpool.tile([128, CH // 128], F32, tag="bv")
            masks_s = wpool.tile([128, 4 * 1024], BF16, tag="masks")
            # first QK-proj accumulation chain consumes (wqkT[ct], xT[ct]) in
            # ct order; interleave the loads (first 512-token slice of each
            # xT tile first) so PE can start a couple of us in instead of
            # after the whole load phase
            nc.sync.dma_start(bqk_s[:], bqk[:, :])
            for i in range(NCT):
                nc.scalar.dma_start(wqkT_s[i][:], wqkT[i * 128:(i + 1) * 128, :])
                nc.sync.dma_start(xT_s[i][:, 0:512], xT[i * 128:(i + 1) * 128, 0:512])
            for i in range(NCT):
                (nc.sync if i % 2 else nc.scalar).dma_start(
                    xT_s[i][:, 512:], xT[i * 128:(i + 1) * 128, 512:])
            for i in range(NCT):
                (nc.scalar if i % 2 else nc.sync).dma_start(
                    wvT_s[i][:], wvT[i * 128:(i + 1) * 128, :])
            nc.scalar.dma_start(masks_s[:], masks[:, :])
            nc.sync.dma_start(bv_s[:], bv[:, :])
            for i in range(CH // 128):
                nc.sync.dma_start(wpT_s[i][:], wpT[i * 128:(i + 1) * 128, :])
            # pre-touch bqk on DVE early (single-wait discipline for the
            # first bias copy); bv/masks are touched late enough naturally
            scratch = wpool.tile([128, 4], F32, tag="scratch")
            nc.vector.tensor_copy(scratch[:, 0:1], bqk_s[:, 0:1])

            # qkT rows: tiles 0..2 = Q^T (384 rows), 3..5 = K^T
            qkT_s = [apool.tile([128, T], BF16, tag=f"qkT{i}", name=f"qkT{i}") for i in range(NCT)]
            v_s = [apool.tile([128, HH * VW], BF16, tag=f"v{i}", name=f"v{i}") for i in range(NTT)]
            yT_s = [apool.tile([128, T], BF16, tag=f"yT{i}", name=f"yT{i}") for i in range(CH // 128)]

            # ---- QK^T projection (emitted per head-pair, interleaved
            # with attention so ACT starts early) ----
            def qk_proj(ot):
                for tcn in range(NQC):
                    ps = ps2.tile([128, 512], F32, tag="mm", name="ps")
                    for ct in range(NCT):
                        nc.tensor.matmul(
                            ps[:],
                            lhsT=wqkT_s[ct][:, ot * 128:(ot + 1) * 128],
                            rhs=xT_s[ct][:, tcn * 512:(tcn + 1) * 512],
                            start=(ct == 0),
                            stop=(ct == NCT - 1),
                        )
                    nc.vector.tensor_scalar_add(
                        qkT_s[ot][:, tcn * 512:(tcn + 1) * 512], ps[:],
                        bqk_s[:, ot:ot + 1],
                    )

            def v_proj(tt):
                ps = ps2.tile([128, CH], F32, tag="mm", name="ps")
                for ct in range(NCT):
                    nc.tensor.matmul(
                        ps[:],
                        lhsT=xT_s[ct][:, tt * 128:(tt + 1) * 128],
                        rhs=wvT_s[ct][:],
                        start=(ct == 0),
                        stop=(ct == NCT - 1),
                    )
                v3 = v_s[tt][:].rearrange("p (h w) -> p h w", w=VW)
                nc.vector.tensor_copy(
                    v3[:, :, 0:64], ps[:].rearrange("p (h d) -> p h d", d=64)
                )
                nc.vector.memset(v3[:, :, 64:65], 1.0)

            def proj_out(tcn):
                for ot in range(NCT):
                    ps = ps2.tile([128, 512], F32, tag="mm", name="ps")
                    for ct in range(CH // 128):
                        nc.tensor.matmul(
                            ps[:],
                            lhsT=wpT_s[ct][:, ot * 128:(ot + 1) * 128],
                            rhs=yT_s[ct][:, tcn * 512:(tcn + 1) * 512],
                            start=(ct == 0),
                            stop=(ct == CH // 128 - 1),
                        )
                    so = opool.tile([128, 512], F32, tag="so", name="so")
                    nc.scalar.copy(so[:], ps[:])
                    nc.sync.dma_start(
                        out[ot * 128:(ot + 1) * 128, tcn * 512:(tcn + 1) * 512],
                        so[:],
                    )

            def attention_block(hp, qc):
                    qt = hp      # Q^T rows for heads 2hp,2hp+1 live in tile hp
                    ktile = 3 + hp
                    o_acc = [pacc.tile([65, 512], F32, tag="oacc", name="oacc")
                             for _ in range(2)]
                    nkt = 4 * (qc + 1)
                    pend = None  # software pipeline: O(kt) issued after S(kt+1)

                    def emit_o(p2, kt, w):
                        for hi in range(2):
                            h = 2 * hp + hi
                            nc.tensor.matmul(
                                o_acc[hi][:, w:512],
                                lhsT=v_s[kt][:, h * VW:(h + 1) * VW],
                                rhs=p2[:, hi * 512 + w:(hi + 1) * 512],
                                start=(kt == 0),
                                stop=(kt == nkt - 1),
                            )

                    for kt in range(nkt):
                        j = kt - 4 * qc
                        # columns q < j*128 of a diagonal block are fully
                        # masked: S, exp, mask and O all skip them (the first
                        # O matmul, start=True, is always full width)
                        w = j * 128 if j >= 1 else 0
                        s2 = ps2.tile([128, 1024], F32, tag="s2", name="s2")
                        for hi in range(2):
                            base = hi * 64
                            nc.tensor.matmul(
                                s2[:, hi * 512 + w:(hi + 1) * 512],
                                lhsT=qkT_s[ktile][base:base + 64,
                                                  kt * 128:(kt + 1) * 128],
                                rhs=qkT_s[qt][base:base + 64,
                                              qc * 512 + w:(qc + 1) * 512],
                                start=True, stop=True,
                            )
                        if pend is not None:
                            emit_o(*pend)
                        p2 = ppool.tile([128, 1024], BF16, tag="pt", name="p2")
                        p3 = p2[:].rearrange("p (c q) -> p c q", c=2)
                        s3 = s2[:].rearrange("p (c q) -> p c q", c=2)
                        nc.scalar.activation(
                            p3[:, :, w:512], s3[:, :, w:512], AF.Exp,
                            scale=0.125,
                        )
                        if j >= 0:
                            m3 = masks_s[:, j * 1024:(j + 1) * 1024].rearrange(
                                "p (c q) -> p c q", c=2)
                            nc.vector.tensor_mul(
                                p3[:, :, w:512], p3[:, :, w:512], m3[:, :, w:512]
                            )
                        pend = (p2, kt, w)
                    emit_o(*pend)
                    # normalize -> y^T; copy accumulator out first (single
                    # DVE op) so the PSUM bank frees immediately
                    for hi in range(2):
                        base = hi * 64
                        dn = spool.tile([1, 512], F32, tag="dn", name="dn")
                        nc.vector.tensor_copy(dn[:], o_acc[hi][64:65, :])
                        ob = spool.tile([64, 512], F32, tag="ob", name="ob")
                        nc.vector.tensor_copy(ob[:], o_acc[hi][0:64, :])
                        rn = spool.tile([1, 512], F32, tag="rn", name="rn")
                        nc.vector.reciprocal_approx_fast(rn[:], dn[:])
                        rc = spool.tile([64, 512], F32, tag="rc", name="rc")
                        nc.gpsimd.partition_broadcast(rc[:], rn[:], channels=64)
                        ysl = yT_s[hp][base:base + 64, qc * 512:(qc + 1) * 512]
                        nc.vector.tensor_mul(ysl, ob[:], rc[:])
                        nc.vector.tensor_scalar_add(
                            ysl, ysl, bv_s[base:base + 64, hp:hp + 1]
                        )

            qk_proj(0)
            qk_proj(3)
            for qc in range(NQC):
                for tt in range(4 * qc, 4 * qc + 4):
                    v_proj(tt)
                attention_block(0, qc)
            qk_proj(1)
            qk_proj(4)
            for qc in range(NQC):
                attention_block(1, qc)
            qk_proj(2)
            qk_proj(5)
            for qc in (3, 2, 1, 0):
                attention_block(2, qc)
                proj_out(qc)
    nc.compile()
    return nc


_CACHE: dict = {}


def _get_graph():
    if "nc" not in _CACHE:
        _CACHE["nc"] = _build_graph()
    return _CACHE["nc"]


def _bf16(a):
    return np.ascontiguousarray(a.astype(ml_dtypes.bfloat16))


def _make_masks():
    k = np.arange(128)[:, None]
    q = np.arange(512)[None, :]
    m = np.zeros((128, 4 * 1024), np.float32)
    for j in range(4):
        pat = (q >= k + j * 128).astype(np.float32)
        m[:, j * 1024:j * 1024 + 512] = pat
        m[:, j * 1024 + 512:(j + 1) * 1024] = pat
    return _bf16(m)


def _prepare_in_maps(x, W_attn, b_attn, W_proj):
    masks = _make_masks()
    in_maps = []
    for core in range(NCORES):
        b, hh = core // 2, core % 2
        sl = slice(hh * CH, (hh + 1) * CH)
        wq = W_attn[0 * C:1 * C][sl]          # [384, 768]
        wk = W_attn[1 * C:2 * C][sl]
        wv = W_attn[2 * C:3 * C][sl]
        bq = b_attn[0 * C:1 * C][sl]
        bk = b_attn[1 * C:2 * C][sl]
        bvv = b_attn[2 * C:3 * C][sl]
        in_maps.append({
            "xT": _bf16(x[b].T),                                   # [768, 2048]
            "wqkT": _bf16(np.concatenate([wq, wk], 0).T),          # [768, 768]
            "bqk": np.ascontiguousarray(
                np.concatenate([bq, bk]).reshape(-1, 128).T),      # [128, 6]
            "wvT": _bf16(wv.T),                                    # [768, 384]
            "bv": np.ascontiguousarray(bvv.reshape(-1, 128).T),    # [128, 3]
            "wpT": _bf16(W_proj[:, sl].T),                         # [384, 768]
            "masks": masks,
        })
    return in_maps


def _unshard(outs, b_proj):
    y = np.empty((B, T, C), np.float32)
    for b in range(B):
        y[b] = (outs[2 * b]["out"] + outs[2 * b + 1]["out"]).T + b_proj
    return y


def run(x, W_attn, b_attn, W_proj, b_proj, **spmd_kwargs):
    x = np.asarray(x, np.float32)
    W_attn = np.asarray(W_attn, np.float32)
    b_attn = np.asarray(b_attn, np.float32)
    W_proj = np.asarray(W_proj, np.float32)
    b_proj = np.asarray(b_proj, np.float32)
    in_maps = _prepare_in_maps(x, W_attn, b_attn, W_proj)
    nc = _get_graph()
    res = run_bass_kernel_spmd(
        nc, in_maps, core_ids=list(range(NCORES)), **spmd_kwargs
    )
    return _unshard(res.results, b_proj), res


def kernel(x, W_attn, b_attn, W_proj, b_proj):
    y, _ = run(x, W_attn, b_attn, W_proj, b_proj)
    return y


# revision 47
# speedup vs baseline: 1.1999x; 1.1999x over previous
"""Distributed causal multi-head attention block for 8 TRN2 NeuronCores.

Sharding: core i -> (batch b = i//2, head-half hh = i%2).  Each core computes
attention for 6 of the 12 heads of one batch element, then a row-sharded
c_proj (its 384 input channels -> full 768 outputs, partial sums).  The
host sums the two partial projections per batch (the "all-reduce" of the
tensor-parallel c_proj) and adds b_proj.

Everything on-chip lives transposed ([feature, token]) so no transposes are
needed:
  qkT = W_qk @ x^T          (heads' Q^T,K^T in [d, t] layout)
  V   = x @ Wv^T            ([t, d] layout, + per-head ones column)
  S^T = K_blk @ Q_blk^T     -> exp (scale 1/8 folded in) -> causal mask
  O^T_aug = [V|1]^T @ P^T   (row 64 of each head block = softmax denom)
  y^T = O^T * (1/denom)  + bv
  out^T = Wp_half @ y^T     (partial over this core's heads)
"""

import sys

sys.path.insert(0, "/opt/trn_rl_repo")

import numpy as np
import ml_dtypes

import concourse.bass as bass
import concourse.bacc as bacc
import concourse.mybir as mybir
import concourse.tile as tile
from concourse.bass_utils import run_bass_kernel_spmd

BF16 = mybir.dt.bfloat16
F32 = mybir.dt.float32
F32R = mybir.dt.float32r
AF = mybir.ActivationFunctionType
ALU = mybir.AluOpType

B, T, C, H, HD = 4, 2048, 768, 12, 64
NCORES = 8
HH = 6              # heads per core
CH = HH * HD        # 384 channels per core
NCT = C // 128      # 6 contraction tiles over C
NTT = T // 128      # 16 token tiles
NQC = T // 512      # 4 query chunks
VW = 65             # per-head V block width (64 dims + ones column)


def _build_graph():
    nc = bacc.Bacc("TRN2", target_bir_lowering=False)

    xT = nc.declare_dram_parameter("xT", [C, T], BF16, isOutput=False)
    wqkT = nc.declare_dram_parameter("wqkT", [C, 2 * CH], BF16, isOutput=False)
    bqk = nc.declare_dram_parameter("bqk", [128, 2 * CH // 128], F32, isOutput=False)
    wvT = nc.declare_dram_parameter("wvT", [C, CH], BF16, isOutput=False)
    bv = nc.declare_dram_parameter("bv", [128, CH // 128], F32, isOutput=False)
    wpT = nc.declare_dram_parameter("wpT", [CH, C], BF16, isOutput=False)
    masks = nc.declare_dram_parameter("masks", [128, 4 * 1024], BF16, isOutput=False)
    out = nc.declare_dram_parameter("out", [C, T], F32, isOutput=True)

    with tile.TileContext(nc) as tc:
        with (
            tc.tile_pool(name="weights", bufs=1) as wpool,
            tc.tile_pool(name="acts", bufs=1) as apool,
            tc.tile_pool(name="ps2", bufs=2, space="PSUM") as ps2,
            tc.tile_pool(name="pacc", bufs=2, space="PSUM") as pacc,
            tc.tile_pool(name="ptile", bufs=8) as ppool,
            tc.tile_pool(name="small", bufs=6) as spool,
            tc.tile_pool(name="ostage", bufs=6) as opool,
        ):
            # ---- load everything ----
            xT_s = [wpool.tile([128, T], BF16, tag=f"xT{i}", name=f"xT{i}") for i in range(NCT)]
            wqkT_s = [wpool.tile([128, 2 * CH], BF16, tag=f"wqk{i}", name=f"wqk{i}") for i in range(NCT)]
            wvT_s = [wpool.tile([128, CH], BF16, tag=f"wv{i}", name=f"wv{i}") for i in range(NCT)]
            wpT_s = [wpool.tile([128, C], BF16, tag=f"wp{i}", name=f"wp{i}") for i in range(CH // 128)]
            bqk_s = wpool.tile([128, 2 * CH // 128], F32, tag="bqk")
            bv_s = wpool.tile([128, CH // 128], F32, tag="bv")
            masks_s = wpool.tile([128, 4 * 1024], BF16, tag="masks")
            # first QK-proj accumulation chain consumes (wqkT[ct], xT[ct]) in
            # ct order; interleave the loads (first 512-token slice of each
            # xT tile first) so PE can start a couple of us in instead of
            # after the whole load phase
            nc.sync.dma_start(bqk_s[:], bqk[:, :])
            for i in range(NCT):
                nc.scalar.dma_start(wqkT_s[i][:], wqkT[i * 128:(i + 1) * 128, :])
                nc.sync.dma_start(xT_s[i][:, 0:512], xT[i * 128:(i + 1) * 128, 0:512])
            for i in range(NCT):
                (nc.sync if i % 2 else nc.scalar).dma_start(
                    xT_s[i][:, 512:], xT[i * 128:(i + 1) * 128, 512:])
            for i in range(NCT):
                (nc.scalar if i % 2 else nc.sync).dma_start(
                    wvT_s[i][:], wvT[i * 128:(i + 1) * 128, :])
            nc.scalar.dma_start(masks_s[:], masks[:, :])
            nc.sync.dma_start(bv_s[:], bv[:, :])
            for i in range(CH // 128):
                nc.sync.dma_start(wpT_s[i][:], wpT[i * 128:(i + 1) * 128, :])
            # pre-touch bqk on DVE early (single-wait discipline for the
            # first bias copy); bv/masks are touched late enough naturally
            scratch = wpool.tile([128, 4], F32, tag="scratch")
            nc.vector.tensor_copy(scratch[:, 0:1], bqk_s[:, 0:1])

            # qkT rows: tiles 0..2 = Q^T (384 rows), 3..5 = K^T
            qkT_s = [apool.tile([128, T], BF16, tag=f"qkT{i}", name=f"qkT{i}") for i in range(NCT)]
            v_s = [apool.tile([128, HH * VW], BF16, tag=f"v{i}", name=f"v{i}") for i in range(NTT)]
            yT_s = [apool.tile([128, T], BF16, tag=f"yT{i}", name=f"yT{i}") for i in range(CH // 128)]

            # ---- QK^T projection (emitted per head-pair, interleaved
            # with attention so ACT starts early) ----
            def qk_proj(ot):
                for tcn in range(NQC):
                    ps = ps2.tile([128, 512], F32, tag="mm", name="ps")
                    for ct in range(NCT):
                        nc.tensor.matmul(
                            ps[:],
                            lhsT=wqkT_s[ct][:, ot * 128:(ot + 1) * 128],
                            rhs=xT_s[ct][:, tcn * 512:(tcn + 1) * 512],
                            start=(ct == 0),
                            stop=(ct == NCT - 1),
                        )
                    nc.scalar.activation(
                        qkT_s[ot][:, tcn * 512:(tcn + 1) * 512], ps[:],
                        AF.Identity, bias=bqk_s[:, ot:ot + 1], scale=1.0,
                    )

            def v_proj(tt):
                ps = ps2.tile([128, CH], F32, tag="mm", name="ps")
                for ct in range(NCT):
                    nc.tensor.matmul(
                        ps[:],
                        lhsT=xT_s[ct][:, tt * 128:(tt + 1) * 128],
                        rhs=wvT_s[ct][:],
                        start=(ct == 0),
                        stop=(ct == NCT - 1),
                    )
                v3 = v_s[tt][:].rearrange("p (h w) -> p h w", w=VW)
                nc.scalar.copy(
                    v3[:, :, 0:64], ps[:].rearrange("p (h d) -> p h d", d=64)
                )
                nc.vector.memset(v3[:, :, 64:65], 1.0)

            def proj_out(tcn):
                for ot in range(NCT):
                    ps = ps2.tile([128, 512], F32, tag="mm", name="ps")
                    for ct in range(CH // 128):
                        nc.tensor.matmul(
                            ps[:],
                            lhsT=wpT_s[ct][:, ot * 128:(ot + 1) * 128],
                            rhs=yT_s[ct][:, tcn * 512:(tcn + 1) * 512],
                            start=(ct == 0),
                            stop=(ct == CH // 128 - 1),
                        )
                    so = opool.tile([128, 512], F32, tag="so", name="so")
                    nc.scalar.copy(so[:], ps[:])
                    nc.sync.dma_start(
                        out[ot * 128:(ot + 1) * 128, tcn * 512:(tcn + 1) * 512],
                        so[:],
                    )

            def attention_block(hp, qc):
                    qt = hp      # Q^T rows for heads 2hp,2hp+1 live in tile hp
                    ktile = 3 + hp
                    o_acc = [pacc.tile([65, 512], F32, tag="oacc", name="oacc")
                             for _ in range(2)]
                    nkt = 4 * (qc + 1)
                    pend = None  # software pipeline: O(kt) issued after S(kt+1)

                    def emit_o(p2, kt, w):
                        for hi in range(2):
                            h = 2 * hp + hi
                            nc.tensor.matmul(
                                o_acc[hi][:, w:512],
                                lhsT=v_s[kt][:, h * VW:(h + 1) * VW],
                                rhs=p2[:, hi * 512 + w:(hi + 1) * 512],
                                start=(kt == 0),
                                stop=(kt == nkt - 1),
                            )

                    for kt in range(nkt):
                        j = kt - 4 * qc
                        # columns q < j*128 of a diagonal block are fully
                        # masked: S, exp, mask and O all skip them (the first
                        # O matmul, start=True, is always full width)
                        w = j * 128 if j >= 1 else 0
                        s2 = ps2.tile([128, 1024], F32, tag="s2", name="s2")
                        for hi in range(2):
                            base = hi * 64
                            nc.tensor.matmul(
                                s2[:, hi * 512 + w:(hi + 1) * 512],
                                lhsT=qkT_s[ktile][base:base + 64,
                                                  kt * 128:(kt + 1) * 128],
                                rhs=qkT_s[qt][base:base + 64,
                                              qc * 512 + w:(qc + 1) * 512],
                                start=True, stop=True,
                            )
                        if pend is not None:
                            emit_o(*pend)
                        p2 = ppool.tile([128, 1024], BF16, tag="pt", name="p2")
                        p3 = p2[:].rearrange("p (c q) -> p c q", c=2)
                        s3 = s2[:].rearrange("p (c q) -> p c q", c=2)
                        nc.scalar.activation(
                            p3[:, :, w:512], s3[:, :, w:512], AF.Exp,
                            scale=0.125,
                        )
                        if j >= 0:
                            m3 = masks_s[:, j * 1024:(j + 1) * 1024].rearrange(
                                "p (c q) -> p c q", c=2)
                            nc.vector.tensor_mul(
                                p3[:, :, w:512], p3[:, :, w:512], m3[:, :, w:512]
                            )
                        pend = (p2, kt, w)
                    emit_o(*pend)
                    # normalize -> y^T; copy accumulator out first (single
                    # DVE op) so the PSUM bank frees immediately
                    for hi in range(2):
                        base = hi * 64
                        dn = spool.tile([1, 512], F32, tag="dn", name="dn")
                        nc.vector.tensor_copy(dn[:], o_acc[hi][64:65, :])
                        ob = spool.tile([64, 512], F32, tag="ob", name="ob")
                        nc.vector.tensor_copy(ob[:], o_acc[hi][0:64, :])
                        rn = spool.tile([1, 512], F32, tag="rn", name="rn")
                        nc.vector.reciprocal_approx_fast(rn[:], dn[:])
                        rc = spool.tile([64, 512], F32, tag="rc", name="rc")
                        nc.gpsimd.partition_broadcast(rc[:], rn[:], channels=64)
                        ysl = yT_s[hp][base:base + 64, qc * 512:(qc + 1) * 512]
                        nc.vector.tensor_mul(ysl, ob[:], rc[:])
                        nc.vector.tensor_scalar_add(
                            ysl, ysl, bv_s[base:base + 64, hp:hp + 1]
                        )

            qk_proj(0)
            qk_proj(3)
            for qc in range(NQC):
                for tt in range(4 * qc, 4 * qc + 4):
                    v_proj(tt)
                attention_block(0, qc)
            qk_proj(1)
            qk_proj(4)
            for qc in range(NQC):
                attention_block(1, qc)
            qk_proj(2)
            qk_proj(5)
            for qc in (3, 2, 1, 0):
                attention_block(2, qc)
                proj_out(qc)
    nc.compile()
    return nc


_CACHE: dict = {}


def _get_graph():
    if "nc" not in _CACHE:
        _CACHE["nc"] = _build_graph()
    return _CACHE["nc"]


def _bf16(a):
    return np.ascontiguousarray(a.astype(ml_dtypes.bfloat16))


def _make_masks():
    k = np.arange(128)[:, None]
    q = np.arange(512)[None, :]
    m = np.zeros((128, 4 * 1024), np.float32)
    for j in range(4):
        pat = (q >= k + j * 128).astype(np.float32)
        m[:, j * 1024:j * 1024 + 512] = pat
        m[:, j * 1024 + 512:(j + 1) * 1024] = pat
    return _bf16(m)


def _prepare_in_maps(x, W_attn, b_attn, W_proj):
    masks = _make_masks()
    in_maps = []
    for core in range(NCORES):
        b, hh = core // 2, core % 2
        sl = slice(hh * CH, (hh + 1) * CH)
        wq = W_attn[0 * C:1 * C][sl]          # [384, 768]
        wk = W_attn[1 * C:2 * C][sl]
        wv = W_attn[2 * C:3 * C][sl]
        bq = b_attn[0 * C:1 * C][sl]
        bk = b_attn[1 * C:2 * C][sl]
        bvv = b_attn[2 * C:3 * C][sl]
        in_maps.append({
            "xT": _bf16(x[b].T),                                   # [768, 2048]
            "wqkT": _bf16(np.concatenate([wq, wk], 0).T),          # [768, 768]
            "bqk": np.ascontiguousarray(
                np.concatenate([bq, bk]).reshape(-1, 128).T),      # [128, 6]
            "wvT": _bf16(wv.T),                                    # [768, 384]
            "bv": np.ascontiguousarray(bvv.reshape(-1, 128).T),    # [128, 3]
            "wpT": _bf16(W_proj[:, sl].T),                         # [384, 768]
            "masks": masks,
        })
    return in_maps


def _unshard(outs, b_proj):
    y = np.empty((B, T, C), np.float32)
    for b in range(B):
        y[b] = (outs[2 * b]["out"] + outs[2 * b + 1]["out"]).T + b_proj
    return y


def run(x, W_attn, b_attn, W_proj, b_proj, **spmd_kwargs):
    x = np.asarray(x, np.float32)
    W_attn = np.asarray(W_attn, np.float32)
    b_attn = np.asarray(b_attn, np.float32)
    W_proj = np.asarray(W_proj, np.float32)
    b_proj = np.asarray(b_proj, np.float32)
    in_maps = _prepare_in_maps(x, W_attn, b_attn, W_proj)
    nc = _get_graph()
    res = run_bass_kernel_spmd(
        nc, in_maps, core_ids=list(range(NCORES)), **spmd_kwargs
    )
    return _unshard(res.results, b_proj), res


def kernel(x, W_attn, b_attn, W_proj, b_proj):
    y, _ = run(x, W_attn, b_attn, W_proj, b_proj)
    return y
